# revision 1
# baseline (speedup 1.0000x reference)
"""Trainium2 Bass kernel for nn_MCA_12214886990440 (strip-conv dual-axis attention).

Sharding: data-parallel over batch B=8 across 8 NeuronCores (params replicated).

Per-core math (one batch element, C=64, H=W=128, NH=8, D=8):
  xh = bn1(x); xw = bn2(x)
  sc_h = multi-k strip conv along H (3 kernels presummed into one 21-tap conv)
  sc_w = strip conv along W
  qkv (conv bias folded into qkv bias); attention per head reassociated:
      w_o^T = wk_t @ G_w,  G_w = sum_d hq(d)^T wv(d)   [128x128 Gram]
      h_o^T = hk_t @ G_h,  G_h = sum_d wq(d)^T hv(d)
  y = x * sigmoid(wout@w_o + hout@h_o + b)

All matmuls run as float32r (full-rate fp32). Layout pivots (channel-partition
<-> pixel-partition) go through small internal DRAM tensors ("cp parking").
"""
import sys
sys.path.insert(0, "/opt/trn_rl_repo")

import numpy as np

import concourse.bass as bass
import concourse.tile as tile
from concourse import bacc
from concourse import mybir

B, C, H, W, NH, D = 8, 64, 128, 128, 8, 8
KS = [7, 11, 21]
EPS = 1e-5
PAD = 10          # max k//2
NTAP = 21
HW = H * W        # 16384
PADROWS = H + 2 * PAD  # 148
F32 = mybir.dt.float32
F32R = mybir.dt.float32r
AF = mybir.ActivationFunctionType
ALU = mybir.AluOpType

N_CORES = 8
NCHUNK = 32       # pixel chunks of 512
CH = 512


def _r(ap):
    return ap.bitcast(F32R)


def _kernel_body(tc, a, reps=1):
    nc = tc.nc
    for _rep in range(reps):
        _one_pass(tc, a)


def _one_pass(tc, a):
    nc = tc.nc

    # ---------------- persistent pools ----------------
    dp = tc.alloc_tile_pool(name="dram", bufs=1, space="DRAM")
    wp = tc.alloc_tile_pool(name="wts", bufs=1)
    xcp = tc.alloc_tile_pool(name="xc", bufs=2)
    evp = tc.alloc_tile_pool(name="evac", bufs=2)

    # packed weights: two tiles (column layouts must match _prep_weights)
    w128 = wp.tile([128, 1478], F32R, tag="w128", name="w128")
    nc.sync.dma_start(w128[:], a["w128"])
    w64 = wp.tile([128, 387], F32R, tag="w64", name="w64")
    nc.sync.dma_start(w64[:], a["w64"])
    convw = [w128[:, 0:704], w128[:, 704:1408]]
    projw = w128[:, 1408:1472]
    qkv1b = [w128[:, 1472:1473].bitcast(F32), w128[:, 1473:1474].bitcast(F32)]
    bnab = w128[:, 1474:1478].bitcast(F32)
    qkv1 = [w64[:, 0:128], w64[:, 128:256]]
    qkv2 = [w64[:, 256:320], w64[:, 320:384]]
    qkv2b = [w64[0:64, 384:385].bitcast(F32), w64[0:64, 385:386].bitcast(F32)]
    projb = w64[0:64, 386:387].bitcast(F32)

    # internal DRAM parking for the channel->pixel pivot
    cp_qv = [dp.tile([128, HW], F32, tag=f"cp_qv{i}", name=f"cp_qv{i}") for i in range(2)]
    cp_k = [dp.tile([64, HW], F32, tag=f"cp_k{i}", name=f"cp_k{i}") for i in range(2)]
    cp_s = dp.tile([128, HW], F32R, tag="cp_s", name="cp_s")

    # ---------------- phase 1: BN + convs + qkv ----------------
    scp = tc.alloc_tile_pool(name="sc", bufs=1)
    pp = tc.alloc_tile_pool(name="pad", bufs=1)
    ps_conv = tc.alloc_tile_pool(name="ps_conv", bufs=4, space="PSUM")
    ps_qkv1 = tc.alloc_tile_pool(name="ps_qkv1", bufs=2, space="PSUM")
    ps_qkv2 = tc.alloc_tile_pool(name="ps_qkv2", bufs=2, space="PSUM")

    for br in range(2):  # 0 = h-branch (conv along H), 1 = w-branch (conv along W)
        scb = scp.tile([64, HW], F32R, tag="sc", name=f"sc{br}")
        pad = pp.tile([128, PADROWS * W], F32R, tag="pad")
        prr = pad[:].rearrange("p (h j) -> p h j", j=PADROWS)
        if br == 0:
            # h-major rows of W; pad rows top/bottom; parts 64-127 = 1-row shift
            nc.vector.memset(pad[0:64, 0:PAD * W].bitcast(F32), 0.0)
            nc.vector.memset(pad[0:64, (H + PAD) * W:].bitcast(F32), 0.0)
            nc.vector.memset(pad[64:128, 0:(PAD - 1) * W].bitcast(F32), 0.0)
            nc.vector.memset(pad[64:128, (H + PAD - 1) * W:].bitcast(F32), 0.0)
        else:
            # rows of length 148 (w-padded); parts 64-127 = 1-col shift
            nc.vector.memset(prr[0:64, :, 0:PAD].bitcast(F32), 0.0)
            nc.vector.memset(prr[0:64, :, H + PAD:].bitcast(F32), 0.0)
            nc.vector.memset(prr[64:128, :, 0:PAD - 1].bitcast(F32), 0.0)
            nc.vector.memset(prr[64:128, :, H + PAD - 1:].bitcast(F32), 0.0)

        # BatchNorm (affine) into the padded buffer, 4 h-rows per chunk
        av0 = bnab[0:64, 2 * br:2 * br + 1]
        bv0 = bnab[0:64, 2 * br + 1:2 * br + 2]
        av1 = bnab[64:128, 2 * br:2 * br + 1]
        bv1 = bnab[64:128, 2 * br + 1:2 * br + 2]
        for i in range(NCHUNK):
            xc = xcp.tile([128, CH], F32, tag="xc")
            src = a["x"][:, i * CH:(i + 1) * CH]
            nc.sync.dma_start(xc[0:64, :], src)
            nc.sync.dma_start(xc[64:128, :], src)
            if br == 0:
                d0 = pad[0:64, (PAD + 4 * i) * W:(PAD + 4 * i) * W + CH]
                d1 = pad[64:128, (PAD - 1 + 4 * i) * W:(PAD - 1 + 4 * i) * W + CH]
                s0, s1 = xc[0:64, :], xc[64:128, :]
            else:
                d0 = prr[0:64, 4 * i:4 * i + 4, PAD:PAD + W]
                d1 = prr[64:128, 4 * i:4 * i + 4, PAD - 1:PAD - 1 + W]
                s0 = xc[0:64, :].rearrange("p (h w) -> p h w", w=W)
                s1 = xc[64:128, :].rearrange("p (h w) -> p h w", w=W)
            nc.vector.tensor_scalar(d0, s0, av0, bv0, ALU.mult, ALU.add)
            nc.vector.tensor_scalar(d1, s1, av1, bv1, ALU.mult, ALU.add)

        # conv: per 512-px chunk, 10 tap-pair matmuls (K=128) + 1 single (K=64)
        cw = convw[br]
        for ci in range(NCHUNK):
            ps = ps_conv.tile([64, CH], F32, tag="conv")
            for g in range(10):
                if br == 0:
                    rhs = pad[:, (4 * ci + 2 * g) * W:(4 * ci + 2 * g) * W + CH]
                else:
                    rhs = prr[:, 4 * ci:4 * ci + 4, 2 * g:2 * g + W]
                nc.tensor.matmul(ps[:], cw[:, g * 64:(g + 1) * 64], rhs,
                                 start=(g == 0), stop=False)
            if br == 0:
                rhs = pad[0:64, (4 * ci + 20) * W:(4 * ci + 20) * W + CH]
            else:
                rhs = prr[0:64, 4 * ci:4 * ci + 4, 20:20 + W]
            nc.tensor.matmul(ps[:], cw[0:64, 640:704], rhs,
                             start=False, stop=True)
            nc.scalar.activation(scb[:, ci * CH:(ci + 1) * CH], ps[:], AF.Copy)

        # qkv pass 1: q|v (M=128), contiguous chunks -> cp_qv, (h,w)-major rows
        for ci in range(NCHUNK):
            ps = ps_qkv1.tile([128, CH], F32, tag="qkv1")
            nc.tensor.matmul(ps[:], qkv1[br][0:64, :],
                             scb[:, ci * CH:(ci + 1) * CH],
                             start=True, stop=True)
            ev = evp.tile([128, CH], F32, tag="ev")
            nc.scalar.activation(ev[:], ps[:], AF.Identity, bias=qkv1b[br])
            nc.sync.dma_start(cp_qv[br][:, ci * CH:(ci + 1) * CH], ev[:])

        # qkv pass 2: k (M=64), w-column chunks -> cp_k, (w,h)-major rows
        scr = scb[:].rearrange("p (h w) -> p w h", w=W)
        for ci in range(NCHUNK):
            ps = ps_qkv2.tile([64, CH], F32, tag="qkv2")
            nc.tensor.matmul(ps[:], qkv2[br][0:64, :],
                             scr[:, 4 * ci:4 * ci + 4, :],
                             start=True, stop=True)
            ev = evp.tile([128, CH], F32, tag="ev", name="ev2")[0:64, :]
            nc.scalar.activation(ev[:], ps[:], AF.Identity, bias=qkv2b[br])
            nc.sync.dma_start(cp_k[br][:, ci * CH:(ci + 1) * CH], ev[:])

    # release phase-1 pools (LIFO per space)
    ps_qkv2.release()
    ps_qkv1.release()
    ps_conv.release()
    pp.release()
    scp.release()

    # ---------------- phase 2: attention + projection ----------------
    scp2 = tc.alloc_tile_pool(name="scp2", bufs=1)
    s_cp = scp2.tile([128, HW], F32R, tag="s_cp")
    gsb = tc.alloc_tile_pool(name="gsb", bufs=1)
    g_sb = gsb.tile([128, 16 * 128], F32, tag="g_sb")
    sprq = tc.alloc_tile_pool(name="sprq", bufs=4)
    sprv = tc.alloc_tile_pool(name="sprv", bufs=4)
    sprk = tc.alloc_tile_pool(name="sprk", bufs=4)
    btev = tc.alloc_tile_pool(name="btev", bufs=2)
    sigp = tc.alloc_tile_pool(name="sigp", bufs=2)
    outp = tc.alloc_tile_pool(name="outp", bufs=2)
    ps_g = tc.alloc_tile_pool(name="ps_g", bufs=2, space="PSUM")
    ps_bt = tc.alloc_tile_pool(name="ps_bt", bufs=4, space="PSUM")
    ps_pj = tc.alloc_tile_pool(name="ps_pj", bufs=2, space="PSUM")

    # G matrices: gi=0 -> G_w = sum hq^T wv ; gi=1 -> G_h = sum wq^T hv
    # one batched DMA per (branch, head, tensor): DRAM AP reordered (h, d, w)
    for gi in range(2):
        qsrc = cp_qv[0] if gi == 0 else cp_qv[1]   # q lives in rows 0..64
        vsrc = cp_qv[1] if gi == 0 else cp_qv[0]   # v lives in rows 64..128
        for nh in range(NH):
            c0 = nh * D
            gps = ps_g.tile([128, 128], F32, tag="g")
            qa = sprq.tile([128, D * W], F32, tag="q")
            nc.sync.dma_start(
                qa[:], qsrc[c0:c0 + D, :].rearrange("d (h w) -> h d w", w=W))
            va = sprv.tile([128, D * W], F32, tag="v")
            nc.sync.dma_start(
                va[:], vsrc[64 + c0:64 + c0 + D, :].rearrange("d (h w) -> h d w", w=W))
            for d in range(D):
                nc.tensor.matmul(gps[:], qa[:, d * W:(d + 1) * W],
                                 va[:, d * W:(d + 1) * W],
                                 start=(d == 0), stop=(d == D - 1))
            nc.scalar.activation(
                g_sb[:, (gi * NH + nh) * 128:(gi * NH + nh + 1) * 128],
                gps[:], AF.Copy)

    # B^T: per (branch, head): batched k load, 8 matmuls, evac into a
    # per-head buffer, one DMA to DRAM cp_s rows (h, d, w)-ordered
    for gi in range(2):
        ksrc = cp_k[1] if gi == 0 else cp_k[0]  # w_o uses wk; h_o uses hk
        for nh in range(NH):
            c0 = nh * D
            gref = g_sb[:, (gi * NH + nh) * 128:(gi * NH + nh + 1) * 128]
            ka = sprk.tile([128, D * H], F32, tag="k")
            nc.sync.dma_start(
                ka[:], ksrc[c0:c0 + D, :].rearrange("d (w h) -> w d h", h=H))
            bt = btev.tile([128, D * W], F32R, tag="btv")
            for d in range(D):
                bps = ps_bt.tile([128, 128], F32, tag="bt")
                nc.tensor.matmul(bps[:], ka[:, d * H:(d + 1) * H], gref,
                                 start=True, stop=True)
                nc.scalar.activation(bt[:, d * W:(d + 1) * W], bps[:], AF.Copy)
            nc.sync.dma_start(
                cp_s[gi * 64 + c0:gi * 64 + c0 + D, :].rearrange(
                    "d (h w) -> h d w", w=W),
                bt[:])

    # bring S back to channel-partition SBUF for the projection
    for i in range(8):
        nc.sync.dma_start(s_cp[:, i * 2048:(i + 1) * 2048],
                          cp_s[:, i * 2048:(i + 1) * 2048])

    # collapse the 128 reverse-spread DMA deps into one sync point so the
    # first projection matmul doesn't exceed the per-instruction wait limit
    tc.strict_bb_all_engine_barrier()

    # fused output projection (both branches, K=128) + sigmoid + x*sig -> y
    for ci in range(NCHUNK):
        pps = ps_pj.tile([64, CH], F32, tag="pj")
        nc.tensor.matmul(pps[:], projw, s_cp[:, ci * CH:(ci + 1) * CH],
                         start=True, stop=True)
        sg = sigp.tile([64, CH], F32, tag="sg")
        nc.scalar.activation(sg[:], pps[:], AF.Sigmoid, bias=projb)
        xc = outp.tile([64, CH], F32, tag="xm")
        nc.sync.dma_start(xc[:], a["x"][:, ci * CH:(ci + 1) * CH])
        ot = outp.tile([64, CH], F32, tag="ot")
        nc.vector.tensor_mul(ot[:], sg[:], xc[:])
        nc.sync.dma_start(a["y"][:, ci * CH:(ci + 1) * CH], ot[:])

    for p in (ps_pj, ps_bt, ps_g, outp, sigp, btev, sprk, sprv, sprq,
              gsb, scp2, evp, xcp, wp, dp):
        p.release()


def _prep_weights(inputs):
    """Host-side packing: BN affine, presummed conv taps, folded qkv biases."""
    inp = {k: np.asarray(v, dtype=np.float64) for k, v in inputs.items()}
    w = {}
    a1 = inp["bn1_g"] / np.sqrt(inp["bn1_v"] + EPS)
    b1 = inp["bn1_b"] - inp["bn1_m"] * a1
    a2 = inp["bn2_g"] / np.sqrt(inp["bn2_v"] + EPS)
    b2 = inp["bn2_b"] - inp["bn2_m"] * a2
    w["bnab"] = np.tile(np.stack([a1, b1, a2, b2], axis=1), (2, 1))  # [128, 4]

    def conv_pack(ws):
        eff = np.zeros((NTAP, C, C))
        for j, k in enumerate(KS):
            off = PAD - k // 2
            for i in range(k):
                eff[off + i] += ws[j][:, :, i]
        pk = np.zeros((128, 704))
        for g in range(10):
            pk[0:64, g * 64:(g + 1) * 64] = eff[2 * g].T
            pk[64:128, g * 64:(g + 1) * 64] = eff[2 * g + 1].T
        pk[0:64, 640:704] = eff[20].T
        return pk

    w["convw_h"] = conv_pack([inp[f"sc1_w{j}"][:, :, :, 0] for j in range(3)])
    w["convw_w"] = conv_pack([inp[f"sc2_w{j}"][:, :, 0, :] for j in range(3)])
    bch = inp["sc1_b0"] + inp["sc1_b1"] + inp["sc1_b2"]
    bcw = inp["sc2_b0"] + inp["sc2_b1"] + inp["sc2_b2"]

    scale = D * H ** (-0.5)
    idx = (np.arange(NH)[:, None] * 24 + np.arange(D)[None, :]).ravel()
    idx_q, idx_k, idx_v = idx, idx + 8, idx + 16

    for br, (qw, qb, bc) in enumerate(
            [(inp["hqkv_w"], inp["hqkv_b"], bch),
             (inp["wqkv_w"], inp["wqkv_b"], bcw)]):
        bfold = qb + qw @ bc
        Wq, Wk, Wv = qw[idx_q] * scale, qw[idx_k], qw[idx_v]
        bq, bk, bv = bfold[idx_q] * scale, bfold[idx_k], bfold[idx_v]
        sfx = "h" if br == 0 else "w"
        w[f"qkv1_{sfx}"] = np.concatenate([Wq.T, Wv.T], axis=1)        # [64,128]
        w[f"qkv1b_{sfx}"] = np.concatenate([bq, bv])[:, None]          # [128,1]
        w[f"qkv2_{sfx}"] = Wk.T                                        # [64,64]
        w[f"qkv2b_{sfx}"] = bk[:, None]                                # [64,1]

    w["projw"] = np.concatenate([inp["wout_w"].T, inp["hout_w"].T], axis=0)  # [128,64]
    w["projb"] = (inp["wout_b"] + inp["hout_b"])[:, None]                    # [64,1]

    w128 = np.zeros((128, 1478))
    w128[:, 0:704] = w["convw_h"]
    w128[:, 704:1408] = w["convw_w"]
    w128[:, 1408:1472] = w["projw"]
    w128[:, 1472:1473] = w["qkv1b_h"]
    w128[:, 1473:1474] = w["qkv1b_w"]
    w128[:, 1474:1478] = w["bnab"]
    w64 = np.zeros((64, 387))
    w64[:, 0:128] = w["qkv1_h"]
    w64[:, 128:256] = w["qkv1_w"]
    w64[:, 256:320] = w["qkv2_h"]
    w64[:, 320:384] = w["qkv2_w"]
    w64[:, 384:385] = w["qkv2b_h"]
    w64[:, 385:386] = w["qkv2b_w"]
    w64[:, 386:387] = w["projb"]
    w64 = np.concatenate([w64, w64], axis=0)  # duplicate onto parts 64-127
    return {"w128": _to_f32r(w128), "w64": _to_f32r(w64)}


_NC_CACHE = {}
_RUN_OPTS = {"trace": False}
_LAST_RESULT = {}

_W_SHAPES = {"x": [C, HW], "w128": [128, 1478], "w64": [128, 387]}
_W_DTYPES = {"x": F32, "w128": F32R, "w64": F32R}


def _to_f32r(a):
    """fp32 -> fp32r: round mantissa to 11 bits (top 20 bits kept)."""
    u = np.ascontiguousarray(a, dtype=np.float32).view(np.uint32).astype(np.uint64)
    u = (u + 0x800) & np.uint64(0xFFFFF000)
    return u.astype(np.uint32).view(np.float32)


def _build_nc(reps=1):
    key = f"nc{reps}"
    if key in _NC_CACHE:
        return _NC_CACHE[key]
    nc = bacc.Bacc(trn_type="TRN2", target_bir_lowering=False, debug=False)
    a = {}
    for n, s in _W_SHAPES.items():
        a[n] = nc.dram_tensor(n, s, _W_DTYPES[n], kind="ExternalInput").ap()
    a["y"] = nc.dram_tensor("y", [C, HW], F32, kind="ExternalOutput").ap()
    with tile.TileContext(nc) as tc:
        _kernel_body(tc, a, reps=reps)
    nc.compile()
    _NC_CACHE[key] = nc
    return nc


def _in_maps(inputs):
    w = _prep_weights(inputs)
    x = np.ascontiguousarray(np.asarray(inputs["x"], dtype=np.float32))
    maps = []
    for core in range(N_CORES):
        m = {"x": np.ascontiguousarray(x[core].reshape(C, HW))}
        m.update(w)
        maps.append(m)
    return maps


def kernel(**inputs):
    from concourse.bass_utils import run_bass_kernel_spmd

    nc = _build_nc()
    res = run_bass_kernel_spmd(nc, _in_maps(inputs), core_ids=list(range(N_CORES)),
                               trace=_RUN_OPTS["trace"])
    _LAST_RESULT["res"] = res
    out = np.stack([res.results[i]["y"].reshape(C, H, W) for i in range(N_CORES)])
    return out.astype(np.float32)


if __name__ == "__main__":
    nc = _build_nc()
    print("built ok")



# revision 2
# speedup vs baseline: 1.3141x; 1.3141x over previous
"""Trainium2 Bass kernel v2 for nn_MCA_12214886990440 (strip-conv dual-axis attention).

Sharding: data-parallel over batch B=8 across 8 NeuronCores (params replicated).

All layout pivots are on-chip xbar DMA transposes (fp16), no DRAM parking.
BN is folded into conv weights host-side. Conv runs col-tiled (2 pixel chunks
concurrently on PE column halves); qkv runs row-tiled (2 chunks on the two PE
row halves). Attention is reassociated through 128x128 Grams:
    G_w[w2,w]   = sum_{d,h} hq[d,h,w2] wv[d,h,w]       (scale folded into Wq)
    w_o[w,(d,h)] = sum_{w2} G_w[w2,w] wk[d,h,w2]
and symmetrically for the h-branch. Final: y = x * sigmoid(wout@w_o + hout@h_o).
"""
import sys
sys.path.insert(0, "/opt/trn_rl_repo")

import numpy as np

import concourse.bass as bass
import concourse.tile as tile
from concourse import bacc
from concourse import mybir

B, C, H, W, NH, D = 8, 64, 128, 128, 8, 8
KS = [7, 11, 21]
EPS = 1e-5
PAD = 10
NTAP = 21
HW = H * W
PADROWS = H + 2 * PAD  # 148
F32 = mybir.dt.float32
F16 = mybir.dt.float16
AF = mybir.ActivationFunctionType
ALU = mybir.AluOpType

N_CORES = 8
CH = 512          # pixel chunk
NPAIR = 16        # chunk pairs (ci, ci+16)


DEBUG = False


def _kernel_body(tc, a):
    nc = tc.nc

    # ---------------- pools (alloc order = reverse release order) -----------
    wp = tc.alloc_tile_pool(name="wts", bufs=1)
    zp = tc.alloc_tile_pool(name="z", bufs=1)
    gp = tc.alloc_tile_pool(name="g", bufs=1)
    scp = tc.alloc_tile_pool(name="sc", bufs=1)
    chp = tc.alloc_tile_pool(name="chan", bufs=1)
    pp = tc.alloc_tile_pool(name="pad", bufs=1)

    # weights
    wconv = wp.tile([128, 2 * 704], F16, tag="wconv", name="wconv")
    nc.sync.dma_start(wconv[:], a["wconv"])
    wqkv = wp.tile([128, 448], F16, tag="wqkv", name="wqkv")
    nc.sync.dma_start(wqkv[:], a["wqkv"])
    wbias = wp.tile([128, 9], F32, tag="wbias", name="wbias")
    nc.sync.dma_start(wbias[:], a["wbias"])
    convw = [wconv[:, 0:704], wconv[:, 704:1408]]
    qkv1w = [wqkv[:, 0:128], wqkv[:, 128:256]]
    qkv2w = [wqkv[:, 256:320], wqkv[:, 320:384]]
    projw = wqkv[:, 384:448]
    convb = [wbias[:, 0:1], wbias[:, 1:2]]
    qkv1b = [wbias[:, 2:3], wbias[:, 3:4]]
    qkv2b = [wbias[:, 4:5], wbias[:, 5:6]]
    projb = wbias[:, 6:7]
    nbv = [wbias[:, 7:8], wbias[:, 8:9]]

    # persistent pivoted tensors (fp16)
    zqv = [zp.tile([128, 128, 128], F16, tag=f"zqv{br}", name=f"zqv{br}")
           for br in range(2)]                       # [h, (w, c)]: c 0-63 q, 64-127 v
    zk = [zp.tile([128, 128, 64], F16, tag=f"zk{br}", name=f"zk{br}")
          for br in range(2)]                        # [w, (h, c)]
    gsb = gp.tile([128, 16 * 128], F16, tag="gsb", name="gsb")

    # ---------------- phase A: conv + qkv per branch ----------------
    ps_conv = tc.alloc_tile_pool(name="ps_conv", bufs=2, space="PSUM")
    ps_qkv1 = tc.alloc_tile_pool(name="ps_qkv1", bufs=1, space="PSUM")
    ps_qkv2 = tc.alloc_tile_pool(name="ps_qkv2", bufs=1, space="PSUM")

    for br in range(2):  # 0 = h-branch (conv along H), 1 = w-branch
        sc = scp.tile([128, 16 * CH], F16, tag="sc", name=f"sc{br}")
        cqv = chp.tile([128, HW], F16, tag="cqv", name=f"cqv{br}")
        ck = chp.tile([128, 8192], F16, tag="ck", name=f"ck{br}")

        pad = pp.tile([128, PADROWS * W], F16, tag="pad", name=f"pad{br}")
        prr = pad[:].rearrange("p (h j) -> p h j", j=PADROWS)
        if br == 0:
            # h-major rows; parts 64-127 = copy shifted one h-row earlier.
            # Borders hold -b/a per channel so folded BN gives 0 there.
            nc.gpsimd.memset(pad[0:64, 0:PAD * W], 0.0)
            nc.gpsimd.memset(pad[0:64, (H + PAD) * W:], 0.0)
            nc.gpsimd.memset(pad[64:128, 0:(PAD - 1) * W], 0.0)
            nc.gpsimd.memset(pad[64:128, (H + PAD - 1) * W:], 0.0)
            nc.vector.tensor_scalar_add(pad[0:64, 0:PAD * W],
                                        pad[0:64, 0:PAD * W], nbv[br][0:64, :])
            nc.vector.tensor_scalar_add(pad[0:64, (H + PAD) * W:],
                                        pad[0:64, (H + PAD) * W:],
                                        nbv[br][0:64, :])
            nc.vector.tensor_scalar_add(pad[64:128, 0:(PAD - 1) * W],
                                        pad[64:128, 0:(PAD - 1) * W],
                                        nbv[br][64:128, :])
            nc.vector.tensor_scalar_add(pad[64:128, (H + PAD - 1) * W:],
                                        pad[64:128, (H + PAD - 1) * W:],
                                        nbv[br][64:128, :])
            nc.scalar.dma_start(pad[0:64, PAD * W:(PAD + H) * W], a["x16"])
            nc.scalar.dma_start(pad[64:128, (PAD - 1) * W:(PAD - 1 + H) * W],
                                a["x16"])
        else:
            # rows of length 148 (w-padded); parts 64-127 = 1-col shift
            nc.gpsimd.memset(prr[0:64, :, 0:PAD], 0.0)
            nc.gpsimd.memset(prr[0:64, :, H + PAD:], 0.0)
            nc.gpsimd.memset(prr[64:128, :, 0:PAD - 1], 0.0)
            nc.gpsimd.memset(prr[64:128, :, H + PAD - 1:], 0.0)
            nc.vector.tensor_scalar_add(prr[0:64, :, 0:PAD],
                                        prr[0:64, :, 0:PAD], nbv[br][0:64, :])
            nc.vector.tensor_scalar_add(prr[0:64, :, H + PAD:],
                                        prr[0:64, :, H + PAD:],
                                        nbv[br][0:64, :])
            nc.vector.tensor_scalar_add(prr[64:128, :, 0:PAD - 1],
                                        prr[64:128, :, 0:PAD - 1],
                                        nbv[br][64:128, :])
            nc.vector.tensor_scalar_add(prr[64:128, :, H + PAD - 1:],
                                        prr[64:128, :, H + PAD - 1:],
                                        nbv[br][64:128, :])
            xr = a["x16"].rearrange("c (h w) -> c h w", w=W)
            nc.scalar.dma_start(prr[0:64, :, PAD:PAD + W], xr)
            nc.scalar.dma_start(prr[64:128, :, PAD - 1:PAD - 1 + W], xr)

        # conv: chunk pairs (ci, ci+16) col-tiled on PE column halves.
        # psum parts 0-63 = chunk ci out-chans, parts 64-127 = chunk ci+16.
        cw = convw[br]
        for ci in range(NPAIR):
            psA = ps_conv.tile([128, CH], F32, tag="conv_a")
            psB = ps_conv.tile([128, CH], F32, tag="conv_b")
            for g in range(10):
                w_g = cw[:, g * 64:(g + 1) * 64]
                if br == 0:
                    rA = pad[:, (4 * ci + 2 * g) * W:(4 * ci + 2 * g) * W + CH]
                    rB = pad[:, (4 * ci + 64 + 2 * g) * W:
                             (4 * ci + 64 + 2 * g) * W + CH]
                else:
                    rA = prr[:, 4 * ci:4 * ci + 4, 2 * g:2 * g + W]
                    rB = prr[:, 4 * ci + 64:4 * ci + 68, 2 * g:2 * g + W]
                nc.tensor.matmul(psA[0:64, :], w_g, rA,
                                 start=(g == 0), stop=False)
                nc.tensor.matmul(psB[64:128, :], w_g, rB,
                                 start=(g == 0), stop=False)
            w_g = cw[0:64, 640:704]
            if br == 0:
                rA = pad[0:64, (4 * ci + 20) * W:(4 * ci + 20) * W + CH]
                rB = pad[0:64, (4 * ci + 84) * W:(4 * ci + 84) * W + CH]
            else:
                rA = prr[0:64, 4 * ci:4 * ci + 4, 20:20 + W]
                rB = prr[0:64, 4 * ci + 64:4 * ci + 68, 20:20 + W]
            nc.tensor.matmul(psA[0:64, :], w_g, rA, start=False, stop=True)
            nc.tensor.matmul(psB[64:128, :], w_g, rB, start=False, stop=True)
            nc.scalar.activation(sc[0:64, ci * CH:(ci + 1) * CH], psA[0:64, :],
                                 AF.Identity, bias=convb[br][0:64, :])
            nc.scalar.activation(sc[64:128, ci * CH:(ci + 1) * CH],
                                 psB[64:128, :], AF.Identity,
                                 bias=convb[br][64:128, :])

        # qkv1 (q|v, M=128): w-major pixel streams so the xbar transpose
        # lands h on partitions. Row-tiled K=64 x2 over the two h-halves.
        # cqv layout: [c, (w, h)], h inner 128.
        scrA = sc[0:64, :].rearrange("c (q h w) -> c w (q h)", h=4, w=W)
        scrB = sc[64:128, :].rearrange("c (q h w) -> c w (q h)", h=4, w=W)
        cqr = cqv[:].rearrange("c (w h) -> c w h", h=H)
        for wi in range(16):  # 8 w-columns -> N=512
            psA = ps_qkv1.tile([128, CH], F32, tag="qkv1a")
            psB = ps_qkv1.tile([128, CH], F32, tag="qkv1b")
            nc.tensor.matmul(psA[:], qkv1w[br][0:64, :],
                             scrA[:, 8 * wi:8 * wi + 8, :],
                             start=True, stop=True)
            nc.tensor.matmul(psB[:], qkv1w[br][64:128, :],
                             scrB[:, 8 * wi:8 * wi + 8, :],
                             start=True, stop=True)
            nc.scalar.activation(cqr[:, 8 * wi:8 * wi + 8, 0:64], psA[:],
                                 AF.Identity, bias=qkv1b[br])
            nc.scalar.activation(cqr[:, 8 * wi:8 * wi + 8, 64:128], psB[:],
                                 AF.Identity, bias=qkv1b[br])

        # qkv2 (k, M=64): h-major pixel chunks (w inner 128) so the xbar
        # transpose lands w on partitions. ck parts 0-63: [c, (h 0-63, w)],
        # parts 64-127: [c, (h 64-127, w)].
        for ci in range(NPAIR):
            ps2a = ps_qkv2.tile([128, CH], F32, tag="qkv2a")
            ps2b = ps_qkv2.tile([128, CH], F32, tag="qkv2b")
            nc.tensor.matmul(ps2a[0:64, :], qkv2w[br][0:64, :],
                             sc[0:64, ci * CH:(ci + 1) * CH],
                             start=True, stop=True)
            nc.tensor.matmul(ps2b[64:128, :], qkv2w[br][64:128, :],
                             sc[64:128, ci * CH:(ci + 1) * CH],
                             start=True, stop=True)
            nc.vector.tensor_scalar_add(ck[0:64, ci * CH:(ci + 1) * CH],
                                        ps2a[0:64, :], qkv2b[br][0:64, :])
            nc.vector.tensor_scalar_add(ck[64:128, ci * CH:(ci + 1) * CH],
                                        ps2b[64:128, :], qkv2b[br][64:128, :])

        # pivots: xbar transposes (fp16, on-chip)
        nc.sync.dma_start_transpose(zqv[br][:], cqv[:])
        nc.sync.dma_start_transpose(zk[br][:, 0:64, :], ck[0:64, :])
        nc.sync.dma_start_transpose(zk[br][:, 64:128, :], ck[64:128, :])
        if DEBUG:
            nc.sync.dma_start(a[f"dbg_sc{br}"], sc[:])
            nc.sync.dma_start(a[f"dbg_cqv{br}"], cqv[:])
            nc.sync.dma_start(a[f"dbg_ck{br}"], ck[:])
            nc.sync.dma_start(a[f"dbg_zqv{br}"],
                              zqv[br][:].rearrange("h w c -> h (w c)"))
            nc.sync.dma_start(a[f"dbg_zk{br}"],
                              zk[br][:].rearrange("w h c -> w (h c)"))

    ps_qkv2.release()
    ps_qkv1.release()
    ps_conv.release()
    pp.release()
    chp.release()
    scp.release()

    # ---------------- phase B: attention ----------------
    zsp = tc.alloc_tile_pool(name="zs", bufs=1)
    sp = tc.alloc_tile_pool(name="s", bufs=1)
    rp = tc.alloc_tile_pool(name="ring", bufs=2)
    ps_g = tc.alloc_tile_pool(name="ps_g", bufs=2, space="PSUM")
    ps_bt = tc.alloc_tile_pool(name="ps_bt", bufs=2, space="PSUM")
    ps_pj = tc.alloc_tile_pool(name="ps_pj", bufs=1, space="PSUM")

    zs = zsp.tile([128, 16384], F16, tag="zs", name="zs")   # [w, (h, c)]
    zsr = zs[:].rearrange("w (h c) -> w c h", c=128)
    s_cp = sp.tile([128, 128, 128], F16, tag="scp", name="scp")  # [c, h, w]

    # Grams: gi=0: G_w = sum_d hq^T wv; gi=1: G_h = sum_d wq^T hv
    for gi in range(2):
        zq = zqv[0] if gi == 0 else zqv[1]
        zv = zqv[1] if gi == 0 else zqv[0]
        for n in range(NH):
            gps = ps_g.tile([128, CH], F32, tag="g")
            for d in range(D):
                c = n * D + d
                lhs = zq[:, :, c:c + 1].rearrange("h w e -> h (w e)")
                rhs = zv[:, :, 64 + c:65 + c].rearrange("h w e -> h (w e)")
                nc.tensor.matmul(gps[:, 0:128], lhs, rhs,
                                 start=(d == 0), stop=(d == D - 1))
            nc.scalar.activation(
                gsb[:, (gi * NH + n) * 128:(gi * NH + n + 1) * 128],
                gps[:, 0:128], AF.Copy)

    # B^T: w_o[w, (d, h)] = sum_{w2} G[w2, w] * k[d, h, w2]
    for gi in range(2):
        zkk = zk[1] if gi == 0 else zk[0]   # w_o uses wk; h_o uses hk
        for n in range(NH):
            g_ap = gsb[:, (gi * NH + n) * 128:(gi * NH + n + 1) * 128]
            for j in range(2):
                bps = ps_bt.tile([128, CH], F32, tag="bt")
                rhs = zkk[:, :, n * D + 4 * j:n * D + 4 * j + 4].rearrange(
                    "w h d -> w d h")
                nc.tensor.matmul(bps[:], g_ap, rhs, start=True, stop=True)
                c0 = gi * 64 + n * D + 4 * j
                nc.scalar.activation(zsr[:, c0:c0 + 4, :], bps[:], AF.Copy)

    if DEBUG:
        nc.sync.dma_start(a["dbg_gsb"], gsb[:])
        nc.sync.dma_start(a["dbg_zs"], zs[:])

    # S pivot: [w, (h, c)] -> [c, h, w], 4 h-quarter transposes
    for q in range(4):
        nc.sync.dma_start_transpose(
            s_cp[:, q * 32:(q + 1) * 32, :], zs[:, q * 4096:(q + 1) * 4096])

    if DEBUG:
        nc.sync.dma_start(a["dbg_scp"], s_cp[:].rearrange("c h w -> c (h w)"))

    # projection (col-tiled pairs) + sigmoid + x*sig -> y
    s_flat = s_cp[:].rearrange("c a b -> c (a b)")
    for ci in range(NPAIR):
        ppsA = ps_pj.tile([128, CH], F32, tag="pj_a")
        ppsB = ps_pj.tile([128, CH], F32, tag="pj_b")
        nc.tensor.matmul(ppsA[0:64, :], projw,
                         s_flat[:, ci * CH:(ci + 1) * CH],
                         start=True, stop=True)
        nc.tensor.matmul(ppsB[64:128, :], projw,
                         s_flat[:, (ci + 16) * CH:(ci + 17) * CH],
                         start=True, stop=True)
        sg = rp.tile([128, CH], F32, tag="sg")
        nc.scalar.activation(sg[0:64, :], ppsA[0:64, :], AF.Sigmoid,
                             bias=projb[0:64, :])
        nc.scalar.activation(sg[64:128, :], ppsB[64:128, :], AF.Sigmoid,
                             bias=projb[64:128, :])
        xc = rp.tile([128, CH], F32, tag="xc")
        nc.scalar.dma_start(xc[0:64, :], a["x"][:, ci * CH:(ci + 1) * CH])
        nc.scalar.dma_start(xc[64:128, :],
                            a["x"][:, (ci + 16) * CH:(ci + 17) * CH])
        yt = rp.tile([128, CH], F32, tag="yt")
        nc.vector.tensor_mul(yt[:], sg[:], xc[:])
        nc.sync.dma_start(a["y"][:, ci * CH:(ci + 1) * CH], yt[0:64, :])
        nc.sync.dma_start(a["y"][:, (ci + 16) * CH:(ci + 17) * CH],
                          yt[64:128, :])

    for p in (ps_pj, ps_bt, ps_g, rp, sp, zsp, gp, zp, wp):
        p.release()


def _prep_weights(inputs):
    """Host-side packing: BN folded into conv weights, qkv biases folded."""
    inp = {k: np.asarray(v, dtype=np.float64) for k, v in inputs.items()}
    w = {}
    a1 = inp["bn1_g"] / np.sqrt(inp["bn1_v"] + EPS)
    b1 = inp["bn1_b"] - inp["bn1_m"] * a1
    a2 = inp["bn2_g"] / np.sqrt(inp["bn2_v"] + EPS)
    b2 = inp["bn2_b"] - inp["bn2_m"] * a2

    def conv_pack(ws, ab, bb, bias):
        # eff[t][o, i]; BN: x_bn = a*x + b folded: W' = W*diag(a), b' += sum_t W_t@b
        eff = np.zeros((NTAP, C, C))
        for j, k in enumerate(KS):
            off = PAD - k // 2
            for i in range(k):
                eff[off + i] += ws[j][:, :, i]
        bconv = bias + sum(eff[t] @ bb for t in range(NTAP))
        effs = eff * ab[None, None, :]
        pk = np.zeros((128, 704))
        for g in range(10):
            pk[0:64, g * 64:(g + 1) * 64] = effs[2 * g].T
            pk[64:128, g * 64:(g + 1) * 64] = effs[2 * g + 1].T
        pk[0:64, 640:704] = effs[20].T
        return pk, bconv

    pk_h, bc_h = conv_pack([inp[f"sc1_w{j}"][:, :, :, 0] for j in range(3)],
                           a1, b1, inp["sc1_b0"] + inp["sc1_b1"] + inp["sc1_b2"])
    pk_w, bc_w = conv_pack([inp[f"sc2_w{j}"][:, :, 0, :] for j in range(3)],
                           a2, b2, inp["sc2_b0"] + inp["sc2_b1"] + inp["sc2_b2"])

    scale = D * H ** (-0.5)
    idx = (np.arange(NH)[:, None] * 24 + np.arange(D)[None, :]).ravel()
    idx_q, idx_k, idx_v = idx, idx + 8, idx + 16

    wqkv = np.zeros((128, 448))
    wbias = np.zeros((128, 9))
    wbias[:, 0] = np.tile(bc_h, 2)
    wbias[:, 1] = np.tile(bc_w, 2)
    for br, (qw, qb, bc) in enumerate(
            [(inp["hqkv_w"], inp["hqkv_b"], bc_h),
             (inp["wqkv_w"], inp["wqkv_b"], bc_w)]):
        bfold = qb
        Wq, Wk, Wv = qw[idx_q] * scale, qw[idx_k], qw[idx_v]
        bq, bk, bv = bfold[idx_q] * scale, bfold[idx_k], bfold[idx_v]
        q1 = np.concatenate([Wq.T, Wv.T], axis=1)          # [64, 128]
        wqkv[:, br * 128:(br + 1) * 128] = np.tile(q1, (2, 1))
        wqkv[:, 256 + br * 64:256 + (br + 1) * 64] = np.tile(Wk.T, (2, 1))
        wbias[:, 2 + br] = np.concatenate([bq, bv])
        wbias[:, 4 + br] = np.tile(bk, 2)
    wqkv[:, 384:448] = np.concatenate([inp["wout_w"].T, inp["hout_w"].T],
                                      axis=0)              # [128, 64]
    wbias[:, 6] = np.tile(inp["wout_b"] + inp["hout_b"], 2)
    wbias[:, 7] = np.tile(-b1 / a1, 2)
    wbias[:, 8] = np.tile(-b2 / a2, 2)

    wconv = np.concatenate([pk_h, pk_w], axis=1)           # [128, 1408]
    return {"wconv": wconv.astype(np.float16),
            "wqkv": wqkv.astype(np.float16),
            "wbias": wbias.astype(np.float32)}


_NC_CACHE = {}
_RUN_OPTS = {"trace": False}
_LAST_RESULT = {}

_SHAPES = {"x": ([C, HW], F32), "x16": ([C, HW], F16),
           "wconv": ([128, 1408], F16), "wqkv": ([128, 448], F16),
           "wbias": ([128, 9], F32)}


def _build_nc():
    if "nc" in _NC_CACHE:
        return _NC_CACHE["nc"]
    nc = bacc.Bacc(trn_type="TRN2", target_bir_lowering=False, debug=False)
    a = {}
    for n, (s, dt) in _SHAPES.items():
        a[n] = nc.dram_tensor(n, s, dt, kind="ExternalInput").ap()
    a["y"] = nc.dram_tensor("y", [C, HW], F32, kind="ExternalOutput").ap()
    if _kernel_body.__globals__["DEBUG"]:
        dbg = {"dbg_sc0": [128, 8192], "dbg_sc1": [128, 8192],
               "dbg_cqv0": [128, HW], "dbg_cqv1": [128, HW],
               "dbg_ck0": [128, 8192], "dbg_ck1": [128, 8192],
               "dbg_zqv0": [128, HW], "dbg_zqv1": [128, HW],
               "dbg_zk0": [128, 8192], "dbg_zk1": [128, 8192],
               "dbg_gsb": [128, 2048], "dbg_zs": [128, HW],
               "dbg_scp": [128, HW]}
        for n, s in dbg.items():
            a[n] = nc.dram_tensor(n, s, F16, kind="ExternalOutput").ap()
    with tile.TileContext(nc) as tc:
        _kernel_body(tc, a)
    nc.compile()
    _NC_CACHE["nc"] = nc
    return nc


def _in_maps(inputs):
    w = _prep_weights(inputs)
    x = np.ascontiguousarray(np.asarray(inputs["x"], dtype=np.float32))
    maps = []
    for core in range(N_CORES):
        xc = np.ascontiguousarray(x[core].reshape(C, HW))
        m = {"x": xc, "x16": xc.astype(np.float16)}
        m.update(w)
        maps.append(m)
    return maps


def kernel(**inputs):
    from concourse.bass_utils import run_bass_kernel_spmd

    nc = _build_nc()
    res = run_bass_kernel_spmd(nc, _in_maps(inputs), core_ids=list(range(N_CORES)),
                               trace=_RUN_OPTS["trace"])
    _LAST_RESULT["res"] = res
    out = np.stack([res.results[i]["y"].reshape(C, H, W) for i in range(N_CORES)])
    return out.astype(np.float32)


if __name__ == "__main__":
    nc = _build_nc()
    print("built ok")


# revision 4
# speedup vs baseline: 1.6469x; 1.2533x over previous
"""Trainium2 Bass kernel v2 for nn_MCA_12214886990440 (strip-conv dual-axis attention).

Sharding: data-parallel over batch B=8 across 8 NeuronCores (params replicated).

All layout pivots are on-chip xbar DMA transposes (fp16), no DRAM parking.
BN is folded into conv weights host-side. Conv runs col-tiled (2 pixel chunks
concurrently on PE column halves); qkv runs row-tiled (2 chunks on the two PE
row halves). Attention is reassociated through 128x128 Grams:
    G_w[w2,w]   = sum_{d,h} hq[d,h,w2] wv[d,h,w]       (scale folded into Wq)
    w_o[w,(d,h)] = sum_{w2} G_w[w2,w] wk[d,h,w2]
and symmetrically for the h-branch. Final: y = x * sigmoid(wout@w_o + hout@h_o).
"""
import sys
sys.path.insert(0, "/opt/trn_rl_repo")

import numpy as np

import concourse.bass as bass
import concourse.tile as tile
from concourse import bacc
from concourse import mybir

B, C, H, W, NH, D = 8, 64, 128, 128, 8, 8
KS = [7, 11, 21]
EPS = 1e-5
PAD = 10
NTAP = 21
HW = H * W
PADROWS = H + 2 * PAD  # 148
F32 = mybir.dt.float32
F16 = mybir.dt.float16
AF = mybir.ActivationFunctionType
ALU = mybir.AluOpType

N_CORES = 8
CH = 512          # pixel chunk
NPAIR = 16        # chunk pairs (ci, ci+16)


DEBUG = False


def _kernel_body(tc, a):
    nc = tc.nc

    # ---------------- pools (alloc order = reverse release order) -----------
    wp = tc.alloc_tile_pool(name="wts", bufs=1)
    zp = tc.alloc_tile_pool(name="z", bufs=1)
    gp = tc.alloc_tile_pool(name="g", bufs=1)
    scp = tc.alloc_tile_pool(name="sc", bufs=1)
    chp = tc.alloc_tile_pool(name="chan", bufs=1)
    pp = tc.alloc_tile_pool(name="pad", bufs=1)

    # weights
    wconv = wp.tile([128, 2 * 704], F16, tag="wconv", name="wconv")
    nc.sync.dma_start(wconv[:], a["wconv"])
    wqkv = wp.tile([128, 448], F16, tag="wqkv", name="wqkv")
    nc.sync.dma_start(wqkv[:], a["wqkv"])
    wbias = wp.tile([128, 9], F32, tag="wbias", name="wbias")
    nc.sync.dma_start(wbias[:], a["wbias"])
    convw = [wconv[:, 0:704], wconv[:, 704:1408]]
    qkv1w = [wqkv[:, 0:128], wqkv[:, 128:256]]
    qkv2w = [wqkv[:, 256:320], wqkv[:, 320:384]]
    projw = wqkv[:, 384:448]
    convb = [wbias[:, 0:1], wbias[:, 1:2]]
    qkv1b = [wbias[:, 2:3], wbias[:, 3:4]]
    qkv2b = [wbias[:, 4:5], wbias[:, 5:6]]
    projb = wbias[:, 6:7]
    nbv = [wbias[:, 7:8], wbias[:, 8:9]]

    # persistent pivoted tensors (fp16)
    zqv = [zp.tile([128, 128, 128], F16, tag=f"zqv{br}", name=f"zqv{br}")
           for br in range(2)]                       # [h, (w, c)]: c 0-63 q, 64-127 v
    zk = [zp.tile([128, 128, 64], F16, tag=f"zk{br}", name=f"zk{br}")
          for br in range(2)]                        # [w, (h, c)]
    gsb = gp.tile([128, 16 * 128], F16, tag="gsb", name="gsb")

    # ---------------- phase A: conv + qkv per branch ----------------
    ps_conv = tc.alloc_tile_pool(name="ps_conv", bufs=2, space="PSUM")

    # warm the PE HAM clock-gate while the pad DMAs land (output unused)
    wps = ps_conv.tile([128, CH], F32, tag="conv_a")
    for i in range(20):
        nc.tensor.matmul(wps[:], wconv[:, 0:128], wconv[:, 0:CH],
                         start=(i == 0), stop=(i == 19))
    ps_qkv1 = tc.alloc_tile_pool(name="ps_qkv1", bufs=1, space="PSUM")
    ps_qkv2 = tc.alloc_tile_pool(name="ps_qkv2", bufs=1, space="PSUM")

    for br in range(2):  # 0 = h-branch (conv along H), 1 = w-branch
        sc = scp.tile([128, 16 * CH], F16, tag="sc", name=f"sc{br}")
        # per-chunk pivot sources: xbar transpose needs whole-tile sources
        cqv = [chp.tile([128, 4096], F16, tag=f"cqv{j}", name=f"cqv{br}_{j}")
               for j in range(4)]
        ck = [chp.tile([128, 4096], F16, tag=f"ck{j}", name=f"ck{br}_{j}")
              for j in range(2)]

        pad = pp.tile([128, PADROWS * W], F16, tag="pad", name=f"pad{br}")
        prr = pad[:].rearrange("p (h j) -> p h j", j=PADROWS)
        if br == 0:
            # h-major rows; parts 64-127 = copy shifted one h-row earlier.
            # Borders hold -b/a per channel so folded BN gives 0 there.
            nc.gpsimd.memset(pad[0:64, 0:PAD * W], 0.0)
            nc.gpsimd.memset(pad[0:64, (H + PAD) * W:], 0.0)
            nc.gpsimd.memset(pad[64:128, 0:(PAD - 1) * W], 0.0)
            nc.gpsimd.memset(pad[64:128, (H + PAD - 1) * W:], 0.0)
            nc.vector.tensor_scalar_add(pad[0:64, 0:PAD * W],
                                        pad[0:64, 0:PAD * W], nbv[br][0:64, :])
            nc.vector.tensor_scalar_add(pad[0:64, (H + PAD) * W:],
                                        pad[0:64, (H + PAD) * W:],
                                        nbv[br][0:64, :])
            nc.vector.tensor_scalar_add(pad[64:128, 0:(PAD - 1) * W],
                                        pad[64:128, 0:(PAD - 1) * W],
                                        nbv[br][64:128, :])
            nc.vector.tensor_scalar_add(pad[64:128, (H + PAD - 1) * W:],
                                        pad[64:128, (H + PAD - 1) * W:],
                                        nbv[br][64:128, :])
            for j in range(4):
                r0, r1 = 32 * j, 32 * (j + 1)
                nc.scalar.dma_start(
                    pad[0:64, (PAD + r0) * W:(PAD + r1) * W],
                    a["x16"][:, r0 * W:r1 * W])
                nc.scalar.dma_start(
                    pad[64:128, (PAD - 1 + r0) * W:(PAD - 1 + r1) * W],
                    a["x16"][:, r0 * W:r1 * W])
        else:
            # rows of length 148 (w-padded); parts 64-127 = 1-col shift
            nc.gpsimd.memset(prr[0:64, :, 0:PAD], 0.0)
            nc.gpsimd.memset(prr[0:64, :, H + PAD:], 0.0)
            nc.gpsimd.memset(prr[64:128, :, 0:PAD - 1], 0.0)
            nc.gpsimd.memset(prr[64:128, :, H + PAD - 1:], 0.0)
            nc.vector.tensor_scalar_add(prr[0:64, :, 0:PAD],
                                        prr[0:64, :, 0:PAD], nbv[br][0:64, :])
            nc.vector.tensor_scalar_add(prr[0:64, :, H + PAD:],
                                        prr[0:64, :, H + PAD:],
                                        nbv[br][0:64, :])
            nc.vector.tensor_scalar_add(prr[64:128, :, 0:PAD - 1],
                                        prr[64:128, :, 0:PAD - 1],
                                        nbv[br][64:128, :])
            nc.vector.tensor_scalar_add(prr[64:128, :, H + PAD - 1:],
                                        prr[64:128, :, H + PAD - 1:],
                                        nbv[br][64:128, :])
            xr = a["x16"].rearrange("c (h w) -> c h w", w=W)
            for j in range(4):
                r0, r1 = 32 * j, 32 * (j + 1)
                nc.scalar.dma_start(prr[0:64, r0:r1, PAD:PAD + W],
                                    xr[:, r0:r1, :])
                nc.scalar.dma_start(prr[64:128, r0:r1, PAD - 1:PAD - 1 + W],
                                    xr[:, r0:r1, :])

        # conv: chunk pairs (ci, ci+16) col-tiled on PE column halves.
        # psum parts 0-63 = chunk ci out-chans, parts 64-127 = chunk ci+16.
        cw = convw[br]
        for ci in range(NPAIR):
            psA = ps_conv.tile([128, CH], F32, tag="conv_a")
            psB = ps_conv.tile([128, CH], F32, tag="conv_b")
            for g in range(10):
                w_g = cw[:, g * 64:(g + 1) * 64]
                if br == 0:
                    rA = pad[:, (4 * ci + 2 * g) * W:(4 * ci + 2 * g) * W + CH]
                    rB = pad[:, (4 * ci + 64 + 2 * g) * W:
                             (4 * ci + 64 + 2 * g) * W + CH]
                else:
                    rA = prr[:, 4 * ci:4 * ci + 4, 2 * g:2 * g + W]
                    rB = prr[:, 4 * ci + 64:4 * ci + 68, 2 * g:2 * g + W]
                nc.tensor.matmul(psA[0:64, :], w_g, rA,
                                 start=(g == 0), stop=False)
                nc.tensor.matmul(psB[64:128, :], w_g, rB,
                                 start=(g == 0), stop=False)
            w_g = cw[0:64, 640:704]
            if br == 0:
                rA = pad[0:64, (4 * ci + 20) * W:(4 * ci + 20) * W + CH]
                rB = pad[0:64, (4 * ci + 84) * W:(4 * ci + 84) * W + CH]
            else:
                rA = prr[0:64, 4 * ci:4 * ci + 4, 20:20 + W]
                rB = prr[0:64, 4 * ci + 64:4 * ci + 68, 20:20 + W]
            nc.tensor.matmul(psA[0:64, :], w_g, rA, start=False, stop=True)
            nc.tensor.matmul(psB[64:128, :], w_g, rB, start=False, stop=True)
            nc.scalar.activation(sc[0:64, ci * CH:(ci + 1) * CH], psA[0:64, :],
                                 AF.Identity, bias=convb[br][0:64, :])
            nc.vector.tensor_scalar_add(sc[64:128, ci * CH:(ci + 1) * CH],
                                        psB[64:128, :], convb[br][64:128, :])

        # qkv1 (q|v, M=128): w-major pixel streams so the xbar transpose
        # lands h on partitions. Row-tiled K=64 x2 over the two h-halves.
        # cqv layout: [c, (w, h)], h inner 128.
        scrA = sc[0:64, :].rearrange("c (q h w) -> c w (q h)", h=4, w=W)
        scrB = sc[64:128, :].rearrange("c (q h w) -> c w (q h)", h=4, w=W)
        for wi in range(16):  # 8 w-columns -> N=512
            wq4 = wi // 4
            cqr = cqv[wq4][:].rearrange("c (w h) -> c w h", h=H)
            psA = ps_qkv1.tile([128, CH], F32, tag="qkv1a")
            psB = ps_qkv1.tile([128, CH], F32, tag="qkv1b")
            nc.tensor.matmul(psA[:], qkv1w[br][0:64, :],
                             scrA[:, 8 * wi:8 * wi + 8, :],
                             start=True, stop=True)
            nc.tensor.matmul(psB[:], qkv1w[br][64:128, :],
                             scrB[:, 8 * wi:8 * wi + 8, :],
                             start=True, stop=True)
            wj = wi % 4
            nc.scalar.activation(cqr[:, 8 * wj:8 * wj + 8, 0:64], psA[:],
                                 AF.Identity, bias=qkv1b[br])
            nc.vector.tensor_scalar_add(cqr[:, 8 * wj:8 * wj + 8, 64:128],
                                        psB[:], qkv1b[br])
            if wi % 4 == 3:  # w-chunk of 32 complete -> pivot it now
                nc.sync.dma_start_transpose(
                    zqv[br][:, wq4 * 32:(wq4 + 1) * 32, :], cqv[wq4][:])

        # qkv2 (k, M=64): h-major pixel chunks (w inner 128) so the xbar
        # transpose lands w on partitions. ck parts 0-63: [c, (h 0-63, w)],
        # parts 64-127: [c, (h 64-127, w)].
        for ci in range(NPAIR):
            hj = ci // 8          # h 32-block within each half
            cko = (ci % 8) * CH
            ps2a = ps_qkv2.tile([128, CH], F32, tag="qkv2a")
            ps2b = ps_qkv2.tile([128, CH], F32, tag="qkv2b")
            nc.tensor.matmul(ps2a[0:64, :], qkv2w[br][0:64, :],
                             sc[0:64, ci * CH:(ci + 1) * CH],
                             start=True, stop=True)
            nc.tensor.matmul(ps2b[64:128, :], qkv2w[br][64:128, :],
                             sc[64:128, ci * CH:(ci + 1) * CH],
                             start=True, stop=True)
            nc.vector.tensor_scalar_add(ck[hj][0:64, cko:cko + CH],
                                        ps2a[0:64, :], qkv2b[br][0:64, :])
            nc.scalar.activation(ck[hj][64:128, cko:cko + CH],
                                 ps2b[64:128, :], AF.Identity,
                                 bias=qkv2b[br][64:128, :])
            if ci % 8 == 7:  # h 32-block complete in both halves -> pivot
                nc.sync.dma_start_transpose(
                    zk[br][:, hj * 32:(hj + 1) * 32, :], ck[hj][0:64, :])
                nc.sync.dma_start_transpose(
                    zk[br][:, 64 + hj * 32:64 + (hj + 1) * 32, :],
                    ck[hj][64:128, :])

        if DEBUG:
            nc.sync.dma_start(a[f"dbg_sc{br}"], sc[:])
            nc.sync.dma_start(a[f"dbg_zqv{br}"],
                              zqv[br][:].rearrange("h w c -> h (w c)"))
            nc.sync.dma_start(a[f"dbg_zk{br}"],
                              zk[br][:].rearrange("w h c -> w (h c)"))

    ps_qkv2.release()
    ps_qkv1.release()
    ps_conv.release()
    pp.release()
    chp.release()
    scp.release()

    # ---------------- phase B: attention ----------------
    zsp = tc.alloc_tile_pool(name="zs", bufs=1)
    sp = tc.alloc_tile_pool(name="s", bufs=1)
    rp = tc.alloc_tile_pool(name="ring", bufs=2)
    ps_g = tc.alloc_tile_pool(name="ps_g", bufs=2, space="PSUM")
    ps_bt = tc.alloc_tile_pool(name="ps_bt", bufs=2, space="PSUM")
    ps_pj = tc.alloc_tile_pool(name="ps_pj", bufs=2, space="PSUM")

    zs = zsp.tile([128, 16384], F16, tag="zs", name="zs")   # [w, (h, c)]
    zsr = zs[:].rearrange("w (h c) -> w h c", c=128)
    s_cp = sp.tile([128, 128, 128], F16, tag="scp", name="scp")  # [c, h, w]

    # Grams: gi=0: G_w = sum_d hq^T wv; gi=1: G_h = sum_d wq^T hv
    for gi in range(2):
        zq = zqv[0] if gi == 0 else zqv[1]
        zv = zqv[1] if gi == 0 else zqv[0]
        for n in range(NH):
            gps = ps_g.tile([128, CH], F32, tag="g")
            for d in range(D):
                c = n * D + d
                lhs = zq[:, :, c:c + 1].rearrange("h w e -> h (w e)")
                rhs = zv[:, :, 64 + c:65 + c].rearrange("h w e -> h (w e)")
                nc.tensor.matmul(gps[:, 0:128], lhs, rhs,
                                 start=(d == 0), stop=(d == D - 1))
            gdst = gsb[:, (gi * NH + n) * 128:(gi * NH + n + 1) * 128]
            if n % 2 == 0:
                nc.scalar.activation(gdst, gps[:, 0:128], AF.Copy)
            else:
                nc.vector.tensor_copy(gdst, gps[:, 0:128])

    # B^T: w_o[w, (d, h)] = sum_{w2} G[w2, w] * k[d, h, w2]
    for gi in range(2):
        zkk = zk[1] if gi == 0 else zk[0]   # w_o uses wk; h_o uses hk
        for n in range(NH):
            g_ap = gsb[:, (gi * NH + n) * 128:(gi * NH + n + 1) * 128]
            for j in range(2):
                bps = ps_bt.tile([128, CH], F32, tag="bt")
                rhs = zkk[:, :, n * D + 4 * j:n * D + 4 * j + 4]
                nc.tensor.matmul(bps[:], g_ap, rhs, start=True, stop=True)
                c0 = gi * 64 + n * D + 4 * j
                dst = zsr[:, :, c0:c0 + 4]
                if j == 0:
                    nc.scalar.activation(dst, bps[:], AF.Copy)
                else:
                    nc.vector.tensor_copy(dst, bps[:])

    if DEBUG:
        nc.sync.dma_start(a["dbg_gsb"], gsb[:])
        nc.sync.dma_start(a["dbg_zs"], zs[:])

    # S pivot: [w, (h, c)] -> [c, h, w], one whole-tensor transpose
    nc.sync.dma_start_transpose(s_cp[:], zs[:])

    if DEBUG:
        nc.sync.dma_start(a["dbg_scp"], s_cp[:].rearrange("c h w -> c (h w)"))

    # keep PE warm while the S transposes drain (output unused)
    wps2 = ps_pj.tile([128, CH], F32, tag="pj_a")
    for i in range(16):
        nc.tensor.matmul(wps2[:], gsb[:, 0:128], gsb[:, 0:CH],
                         start=(i == 0), stop=(i == 15))

    # prefetch x (fp16) for the final elementwise multiply
    xpf = rp.tile([128, 8192], F16, tag="xpf")
    nc.sync.dma_start(xpf[0:64, :], a["x16"][:, 0:8192])
    nc.sync.dma_start(xpf[64:128, :], a["x16"][:, 8192:16384])

    # projection (col-tiled pairs) + sigmoid + x*sig -> y
    s_flat = s_cp[:].rearrange("c a b -> c (a b)")
    for ci in range(NPAIR):
        ppsA = ps_pj.tile([128, CH], F32, tag="pj_a")
        ppsB = ps_pj.tile([128, CH], F32, tag="pj_b")
        nc.tensor.matmul(ppsA[0:64, :], projw,
                         s_flat[:, ci * CH:(ci + 1) * CH],
                         start=True, stop=True)
        nc.tensor.matmul(ppsB[64:128, :], projw,
                         s_flat[:, (ci + 16) * CH:(ci + 17) * CH],
                         start=True, stop=True)
        sg = rp.tile([128, CH], F32, tag="sg")
        nc.scalar.activation(sg[0:64, :], ppsA[0:64, :], AF.Sigmoid,
                             bias=projb[0:64, :])
        nc.scalar.activation(sg[64:128, :], ppsB[64:128, :], AF.Sigmoid,
                             bias=projb[64:128, :])
        yt = rp.tile([128, CH], F32, tag="yt")
        nc.vector.tensor_mul(yt[:], sg[:],
                             xpf[:, ci * CH:(ci + 1) * CH])
        nc.scalar.dma_start(a["y"][:, ci * CH:(ci + 1) * CH], yt[0:64, :])
        nc.scalar.dma_start(a["y"][:, (ci + 16) * CH:(ci + 17) * CH],
                            yt[64:128, :])

    for p in (ps_pj, ps_bt, ps_g, rp, sp, zsp, gp, zp, wp):
        p.release()


def _prep_weights(inputs):
    """Host-side packing: BN folded into conv weights, qkv biases folded."""
    inp = {k: np.asarray(v, dtype=np.float64) for k, v in inputs.items()}
    w = {}
    a1 = inp["bn1_g"] / np.sqrt(inp["bn1_v"] + EPS)
    b1 = inp["bn1_b"] - inp["bn1_m"] * a1
    a2 = inp["bn2_g"] / np.sqrt(inp["bn2_v"] + EPS)
    b2 = inp["bn2_b"] - inp["bn2_m"] * a2

    def conv_pack(ws, ab, bb, bias):
        # eff[t][o, i]; BN: x_bn = a*x + b folded: W' = W*diag(a), b' += sum_t W_t@b
        eff = np.zeros((NTAP, C, C))
        for j, k in enumerate(KS):
            off = PAD - k // 2
            for i in range(k):
                eff[off + i] += ws[j][:, :, i]
        bconv = bias + sum(eff[t] @ bb for t in range(NTAP))
        effs = eff * ab[None, None, :]
        pk = np.zeros((128, 704))
        for g in range(10):
            pk[0:64, g * 64:(g + 1) * 64] = effs[2 * g].T
            pk[64:128, g * 64:(g + 1) * 64] = effs[2 * g + 1].T
        pk[0:64, 640:704] = effs[20].T
        return pk, bconv

    pk_h, bc_h = conv_pack([inp[f"sc1_w{j}"][:, :, :, 0] for j in range(3)],
                           a1, b1, inp["sc1_b0"] + inp["sc1_b1"] + inp["sc1_b2"])
    pk_w, bc_w = conv_pack([inp[f"sc2_w{j}"][:, :, 0, :] for j in range(3)],
                           a2, b2, inp["sc2_b0"] + inp["sc2_b1"] + inp["sc2_b2"])

    scale = D * H ** (-0.5)
    idx = (np.arange(NH)[:, None] * 24 + np.arange(D)[None, :]).ravel()
    idx_q, idx_k, idx_v = idx, idx + 8, idx + 16

    wqkv = np.zeros((128, 448))
    wbias = np.zeros((128, 9))
    wbias[:, 0] = np.tile(bc_h, 2)
    wbias[:, 1] = np.tile(bc_w, 2)
    for br, (qw, qb, bc) in enumerate(
            [(inp["hqkv_w"], inp["hqkv_b"], bc_h),
             (inp["wqkv_w"], inp["wqkv_b"], bc_w)]):
        bfold = qb
        Wq, Wk, Wv = qw[idx_q] * scale, qw[idx_k], qw[idx_v]
        bq, bk, bv = bfold[idx_q] * scale, bfold[idx_k], bfold[idx_v]
        q1 = np.concatenate([Wq.T, Wv.T], axis=1)          # [64, 128]
        wqkv[:, br * 128:(br + 1) * 128] = np.tile(q1, (2, 1))
        wqkv[:, 256 + br * 64:256 + (br + 1) * 64] = np.tile(Wk.T, (2, 1))
        wbias[:, 2 + br] = np.concatenate([bq, bv])
        wbias[:, 4 + br] = np.tile(bk, 2)
    wqkv[:, 384:448] = np.concatenate([inp["wout_w"].T, inp["hout_w"].T],
                                      axis=0)              # [128, 64]
    wbias[:, 6] = np.tile(inp["wout_b"] + inp["hout_b"], 2)
    wbias[:, 7] = np.tile(-b1 / a1, 2)
    wbias[:, 8] = np.tile(-b2 / a2, 2)

    wconv = np.concatenate([pk_h, pk_w], axis=1)           # [128, 1408]
    return {"wconv": wconv.astype(np.float16),
            "wqkv": wqkv.astype(np.float16),
            "wbias": wbias.astype(np.float32)}


_NC_CACHE = {}
_RUN_OPTS = {"trace": False}
_LAST_RESULT = {}

_SHAPES = {"x": ([C, HW], F32), "x16": ([C, HW], F16),
           "wconv": ([128, 1408], F16), "wqkv": ([128, 448], F16),
           "wbias": ([128, 9], F32)}


def _build_nc():
    if "nc" in _NC_CACHE:
        return _NC_CACHE["nc"]
    nc = bacc.Bacc(trn_type="TRN2", target_bir_lowering=False, debug=False)
    a = {}
    for n, (s, dt) in _SHAPES.items():
        a[n] = nc.dram_tensor(n, s, dt, kind="ExternalInput").ap()
    a["y"] = nc.dram_tensor("y", [C, HW], F32, kind="ExternalOutput").ap()
    if _kernel_body.__globals__["DEBUG"]:
        dbg = {"dbg_sc0": [128, 8192], "dbg_sc1": [128, 8192],
               "dbg_zqv0": [128, HW], "dbg_zqv1": [128, HW],
               "dbg_zk0": [128, 8192], "dbg_zk1": [128, 8192],
               "dbg_gsb": [128, 2048], "dbg_zs": [128, HW],
               "dbg_scp": [128, HW]}
        for n, s in dbg.items():
            a[n] = nc.dram_tensor(n, s, F16, kind="ExternalOutput").ap()
    with tile.TileContext(nc) as tc:
        _kernel_body(tc, a)
    nc.compile()
    _NC_CACHE["nc"] = nc
    return nc


def _in_maps(inputs):
    w = _prep_weights(inputs)
    x = np.ascontiguousarray(np.asarray(inputs["x"], dtype=np.float32))
    maps = []
    for core in range(N_CORES):
        xc = np.ascontiguousarray(x[core].reshape(C, HW))
        m = {"x": xc, "x16": xc.astype(np.float16)}
        m.update(w)
        maps.append(m)
    return maps


def kernel(**inputs):
    from concourse.bass_utils import run_bass_kernel_spmd

    nc = _build_nc()
    res = run_bass_kernel_spmd(nc, _in_maps(inputs), core_ids=list(range(N_CORES)),
                               trace=_RUN_OPTS["trace"])
    _LAST_RESULT["res"] = res
    out = np.stack([res.results[i]["y"].reshape(C, H, W) for i in range(N_CORES)])
    return out.astype(np.float32)


if __name__ == "__main__":
    nc = _build_nc()
    print("built ok")
